# Initial kernel scaffold
#
"""Trainium2 Bass kernel for nn_Net_5128190952056 (topk_masking).

Data-parallel over batch: 32 samples -> 8 NeuronCores x 4 samples.
Per-core pipeline (all compute on device):
  score = sum_a boxes[:, :, a, 4]  ->  exact top-32 grid selection (DVE
  max8/match_replace hierarchy, jax.top_k-compatible set)  ->  sorted-index
  extraction  ->  indirect-DMA gathers of selected features/boxes  ->
  gated multimodal fusion (PE matmuls)  ->  cosine-sim head  ->  per-sample
  argmax box decode.
"""
import sys
sys.path.insert(0, "/opt/trn_rl_repo")

import numpy as np

import concourse.bass as bass
import concourse.tile as tile
from concourse import bacc, mybir
from concourse.masks import make_identity

F32 = mybir.dt.float32
I32 = mybir.dt.int32
AF = mybir.ActivationFunctionType
OP = mybir.AluOpType

N_CORES = 8
BS = 32            # full batch
B = BS // N_CORES  # per-core batch = 4
GRID = 2704
A = 3
CH = 85
V = 1024
H = 512
SEL = 32
NSEL = B * SEL     # 128
C16 = 16           # score chunks per sample
F169 = GRID // C16  # 169
P64 = B * C16      # 64 partitions for score layout
FLATG = B * GRID   # 10816
KEYC = 32768.0     # key offset for index extraction

# matmul operand dtype: float32 (exact, 4 cyc/row) or float32r (fast, ~1 cyc/row @N>=256)
MM_DT = mybir.dt.float32r


def _mm(ap):
    return ap.bitcast(MM_DT) if MM_DT != F32 else ap


def emit(tc):
    nc = tc.nc

    def dram(name, shape):
        return nc.dram_tensor(name, shape, F32, kind="ExternalInput").ap()

    # ---- external tensors (per-core shapes) ----
    obj_t = dram("obj_t", [B, A, GRID])
    boxes_r = dram("boxes_r", [FLATG, A * CH])
    xf_t = dram("xf_t", [FLATG, V])
    tag_r = dram("tag_r", [NSEL, H])
    pos_r = dram("pos_r", [NSEL, H])
    langT = dram("langT", [H, B])
    W_vs = dram("W_vs", [V, H])
    W_ts = dram("W_ts", [H, H])
    W_vp = dram("W_vp", [H, H])
    W_tg = dram("W_tg", [H, H])
    W_soft = dram("W_soft", [H, 2])
    b_vs = dram("b_vs", [H])
    b_ts = dram("b_ts", [H])
    b_vp = dram("b_vp", [H])
    b_tg = dram("b_tg", [H])
    b_soft = dram("b_soft", [2])
    bsel64 = dram("bsel64", [B, P64])
    bsel128 = dram("bsel128", [B, NSEL])
    out_sim = nc.dram_tensor("out_sim", [B, SEL], F32, kind="ExternalOutput").ap()
    out_box = nc.dram_tensor("out_box", [B, 5], F32, kind="ExternalOutput").ap()
    return _emit_body(tc, locals())


def _emit_body(tc, io):
    nc = tc.nc
    ctx_pools = []

    sb = tc.tile_pool(name="sb", bufs=1).__enter__()
    sb2 = tc.tile_pool(name="sb2", bufs=2).__enter__()
    pbig = tc.tile_pool(name="pbig", bufs=3, space="PSUM").__enter__()
    ptp = tc.tile_pool(name="ptp", bufs=2, space="PSUM").__enter__()
    psm = tc.tile_pool(name="psm", bufs=2, space="PSUM").__enter__()
    dr = tc.tile_pool(name="dr", bufs=1, space="DRAM").__enter__()
    ctx_pools += [sb, sb2, pbig, ptp, psm, dr]

    obj_t, boxes_r, xf_t, tag_r, pos_r, langT = (
        io["obj_t"], io["boxes_r"], io["xf_t"], io["tag_r"], io["pos_r"], io["langT"])
    W_vs, W_ts, W_vp, W_tg, W_soft = (
        io["W_vs"], io["W_ts"], io["W_vp"], io["W_tg"], io["W_soft"])
    b_vs, b_ts, b_vp, b_tg, b_soft = (
        io["b_vs"], io["b_ts"], io["b_vp"], io["b_tg"], io["b_soft"])
    bsel64, bsel128, out_sim, out_box = (
        io["bsel64"], io["bsel128"], io["out_sim"], io["out_box"])

    # =============== phase S: score + exact top-32 selection ===============
    # obj plane [B, A, GRID] -> [64, 3*169]; partition p = 16*s + c
    obj_sb = sb.tile([P64, A * F169], F32)
    nc.sync.dma_start(obj_sb[:], obj_t.rearrange("b a (c f) -> (b c) (a f)", c=C16))

    # weights/constants can stream in parallel (emitted below; Tile reorders by deps)
    s0 = sb.tile([P64, F169], F32)
    nc.vector.tensor_add(s0[:], obj_sb[:, 0:F169], obj_sb[:, F169:2 * F169])
    nc.vector.tensor_add(s0[:], s0[:], obj_sb[:, 2 * F169:3 * F169])

    # per-partition top-32 score candidates
    vt = sb.tile([P64, F169], F32)
    nc.vector.tensor_copy(vt[:], s0[:])
    cand = sb.tile([P64, SEL], F32)
    for r in range(4):
        nc.vector.max(cand[:, 8 * r:8 * r + 8], vt[:])
        nc.vector.match_replace(vt[:], cand[:, 8 * r:8 * r + 8], vt[:], -1.0)

    # merge candidates -> per-sample top-32 values (round-trip through DRAM)
    cvd = dr.tile([P64, SEL], F32)
    nc.scalar.dma_start(cvd[:], cand[:])
    vm = sb.tile([B, C16 * SEL], F32)
    nc.scalar.dma_start(vm[:], cvd[:].rearrange("(s c) j -> s (c j)", s=B))
    gtop = sb.tile([B, SEL], F32)
    for r in range(4):
        nc.vector.max(gtop[:, 8 * r:8 * r + 8], vm[:])
        nc.vector.match_replace(vm[:], gtop[:, 8 * r:8 * r + 8], vm[:], -1.0)

    # threshold t = 32nd largest, broadcast [B,1] -> [64,1] via PE
    bsel64_sb = sb.tile([B, P64], F32)
    nc.sync.dma_start(bsel64_sb[:], bsel64)
    t_ps = psm.tile([P64, 1], F32)
    nc.tensor.matmul(t_ps[:], lhsT=bsel64_sb[:], rhs=gtop[:, SEL - 1:SEL],
                     start=True, stop=True)
    t64 = sb.tile([P64, 1], F32)
    nc.vector.tensor_copy(t64[:], t_ps[:])

    # mask of selected grid cells; key = mask * (KEYC - flat_row_index)
    niota = sb.tile([P64, F169], I32)
    nc.gpsimd.iota(niota[:], pattern=[[-1, F169]], base=int(KEYC),
                   channel_multiplier=-F169)
    nif = sb.tile([P64, F169], F32)
    nc.vector.tensor_copy(nif[:], niota[:])
    mask = sb.tile([P64, F169], F32)
    nc.vector.tensor_scalar(mask[:], s0[:], t64[:, 0:1], None, op0=OP.is_ge)
    key = sb.tile([P64, F169], F32)
    nc.vector.tensor_tensor(key[:], mask[:], nif[:], op=OP.mult)

    kc = sb.tile([P64, SEL], F32)
    for r in range(4):
        nc.vector.max(kc[:, 8 * r:8 * r + 8], key[:])
        nc.vector.match_replace(key[:], kc[:, 8 * r:8 * r + 8], key[:], 0.0)

    kd = dr.tile([P64, SEL], F32)
    nc.scalar.dma_start(kd[:], kc[:])
    km = sb.tile([B, C16 * SEL], F32)
    nc.scalar.dma_start(km[:], kd[:].rearrange("(s c) j -> s (c j)", s=B))
    gk = sb.tile([B, SEL], F32)
    for r in range(4):
        nc.vector.max(gk[:, 8 * r:8 * r + 8], km[:])
        nc.vector.match_replace(km[:], gk[:, 8 * r:8 * r + 8], km[:], 0.0)

    # sorted (ascending) flat row indices of the selected 32 cells per sample
    gidxf = sb.tile([B, SEL], F32)
    nc.vector.tensor_scalar(gidxf[:], gk[:], KEYC, -1.0,
                            op0=OP.subtract, op1=OP.mult)
    gidxi = sb.tile([B, SEL], I32)
    nc.vector.tensor_copy(gidxi[:], gidxf[:])
    gd = dr.tile([NSEL, 1], I32)
    nc.scalar.dma_start(gd[:].rearrange("(s j) one -> s (j one)", s=B), gidxi[:])
    offs = sb.tile([NSEL, 1], I32)
    nc.scalar.dma_start(offs[:], gd[:])

    # =============== phase G: gather selected visual features ===============
    isel = sb.tile([NSEL, V], F32)
    nc.gpsimd.indirect_dma_start(
        out=isel[:], out_offset=None, in_=xf_t,
        in_offset=bass.IndirectOffsetOnAxis(ap=offs[:, :1], axis=0))

    # =============== weights / constants ===============
    wvs_sb = sb.tile([128, 8 * H], F32)
    nc.sync.dma_start(wvs_sb[:], W_vs.rearrange("(k p) h -> p (k h)", p=128))
    wts_sb = sb.tile([128, 4 * H], F32)
    nc.sync.dma_start(wts_sb[:], W_ts.rearrange("(k p) h -> p (k h)", p=128))
    wvp_sb = sb.tile([128, 4 * H], F32)
    nc.sync.dma_start(wvp_sb[:], W_vp.rearrange("(k p) h -> p (k h)", p=128))
    wtg_sb = sb.tile([128, 4 * H], F32)
    nc.sync.dma_start(wtg_sb[:], W_tg.rearrange("(k p) h -> p (k h)", p=128))
    wsoft_sb = sb.tile([1, H * 2], F32)
    nc.sync.dma_start(wsoft_sb[:], W_soft.rearrange("h c -> 1 (h c)"))
    tag_sb = sb.tile([NSEL, H], F32)
    nc.sync.dma_start(tag_sb[:], tag_r)
    pos_sb = sb.tile([NSEL, H], F32)
    nc.sync.dma_start(pos_sb[:], pos_r)
    langT_sb = sb.tile([128, 4 * B], F32)
    nc.sync.dma_start(langT_sb[:], langT.rearrange("(k p) b -> p (k b)", p=128))
    bsel128_sb = sb.tile([B, NSEL], F32)
    nc.sync.dma_start(bsel128_sb[:], bsel128)

    bvs_sb = sb.tile([1, H], F32)
    nc.sync.dma_start(bvs_sb[:], b_vs.rearrange("h -> 1 h"))
    bts_sb = sb.tile([1, H], F32)
    nc.sync.dma_start(bts_sb[:], b_ts.rearrange("h -> 1 h"))
    bvp_sb = sb.tile([1, H], F32)
    nc.sync.dma_start(bvp_sb[:], b_vp.rearrange("h -> 1 h"))
    btg_sb = sb.tile([1, H], F32)
    nc.sync.dma_start(btg_sb[:], b_tg.rearrange("h -> 1 h"))
    bvt_sb = sb.tile([1, H], F32)
    nc.vector.tensor_add(bvt_sb[:], bvp_sb[:], btg_sb[:])
    bsoft_sb = sb.tile([1, 2], F32)
    nc.sync.dma_start(bsoft_sb[:], b_soft.rearrange("c -> 1 c"))

    ones1 = sb.tile([1, 128], F32)
    nc.vector.memset(ones1[:], 1.0)
    ident = sb.tile([128, 128], F32)
    make_identity(nc, ident[:])

    # =============== phase T: transpose gathered features ===============
    itk = sb.tile([128, 8 * 128], F32)
    for k in range(8):
        tp = ptp.tile([128, 128], F32)
        nc.tensor.transpose(out=tp[:], in_=isel[:, 128 * k:128 * (k + 1)],
                            identity=ident[:])
        nc.vector.tensor_copy(itk[:, 128 * k:128 * (k + 1)], tp[:])

    # =============== phase M1: vis = isel @ W_vs + b_vs ===============
    vis_ps = pbig.tile([NSEL, H], F32)
    for k in range(8):
        nc.tensor.matmul(vis_ps[:], lhsT=_mm(itk[:, 128 * k:128 * (k + 1)]),
                         rhs=_mm(wvs_sb[:, H * k:H * (k + 1)]),
                         start=(k == 0), stop=False)
    nc.tensor.matmul(vis_ps[:], lhsT=_mm(ones1[:]), rhs=_mm(bvs_sb[:]),
                     start=False, stop=True)
    vis = sb.tile([NSEL, H], F32)
    nc.vector.tensor_copy(vis[:], vis_ps[:])

    # =============== phase M2: lang_e, broadcast per sample ===============
    le_ps = psm.tile([B, H], F32)
    for k in range(4):
        nc.tensor.matmul(le_ps[:], lhsT=_mm(langT_sb[:, B * k:B * (k + 1)]),
                         rhs=_mm(wts_sb[:, H * k:H * (k + 1)]),
                         start=(k == 0), stop=False)
    nc.tensor.matmul(le_ps[:], lhsT=_mm(ones1[:, 0:B]), rhs=_mm(bts_sb[:]),
                     start=False, stop=True)
    le = sb.tile([B, H], F32)
    nc.vector.tensor_copy(le[:], le_ps[:])
    langB_ps = pbig.tile([NSEL, H], F32)
    nc.tensor.matmul(langB_ps[:], lhsT=_mm(bsel128_sb[:]), rhs=_mm(le[:]),
                     start=True, stop=True)
    langB = sb.tile([NSEL, H], F32)
    nc.vector.tensor_copy(langB[:], langB_ps[:])

    # =============== phase M3: logits + log-softmax weights ===============
    xs = sb.tile([NSEL, H], F32)
    nc.vector.tensor_add(xs[:], vis[:], tag_sb[:])
    nc.vector.tensor_add(xs[:], xs[:], pos_sb[:])

    bs_ps = psm.tile([NSEL, 2], F32)
    nc.tensor.matmul(bs_ps[:], lhsT=_mm(ones1[:]), rhs=_mm(bsoft_sb[:]),
                     start=True, stop=True)
    bs128 = sb.tile([NSEL, 2], F32)
    nc.vector.tensor_copy(bs128[:], bs_ps[:])

    wsb = []
    for c in range(2):
        w_ps = pbig.tile([NSEL, H], F32)
        nc.tensor.matmul(w_ps[:], lhsT=_mm(ones1[:]),
                         rhs=_mm(wsoft_sb[0:1, c:2 * H:2]), start=True, stop=True)
        w_sb = sb.tile([NSEL, H], F32)
        nc.vector.tensor_copy(w_sb[:], w_ps[:])
        wsb.append(w_sb)

    junk = sb2.tile([NSEL, H], F32)
    lg0 = sb.tile([NSEL, 1], F32)
    nc.vector.tensor_tensor_reduce(
        out=junk[:], in0=xs[:], in1=wsb[0][:], scale=1.0,
        scalar=bs128[:, 0:1], op0=OP.mult, op1=OP.add, accum_out=lg0[:])
    junk2 = sb2.tile([NSEL, H], F32)
    lg1 = sb.tile([NSEL, 1], F32)
    nc.vector.tensor_tensor_reduce(
        out=junk2[:], in0=xs[:], in1=wsb[1][:], scale=1.0,
        scalar=bs128[:, 1:2], op0=OP.mult, op1=OP.add, accum_out=lg1[:])

    d = sb.tile([NSEL, 1], F32)
    nc.vector.tensor_sub(d[:], lg0[:], lg1[:])
    # w0 = -softplus(-d/0.03), w1 = -softplus(d/0.03)
    w0 = sb.tile([NSEL, 1], F32)
    nc.scalar.activation(w0[:], d[:], AF.Softplus, scale=-1.0 / 0.03)
    nc.vector.tensor_scalar_mul(w0[:], w0[:], -1.0)
    w1 = sb.tile([NSEL, 1], F32)
    nc.scalar.activation(w1[:], d[:], AF.Softplus, scale=1.0 / 0.03)
    nc.vector.tensor_scalar_mul(w1[:], w1[:], -1.0)

    aS = sb.tile([NSEL, H], F32)
    nc.vector.tensor_scalar(aS[:], vis[:], w0[:, 0:1], None, op0=OP.mult)
    bS = sb.tile([NSEL, H], F32)
    nc.vector.tensor_scalar(bS[:], tag_sb[:], w1[:, 0:1], None, op0=OP.mult)

    # =============== phase T2 + M4: vis2 ===============
    aT = sb.tile([128, 4 * 128], F32)
    bT = sb.tile([128, 4 * 128], F32)
    for src, dst in ((aS, aT), (bS, bT)):
        for k in range(4):
            tp = ptp.tile([128, 128], F32)
            nc.tensor.transpose(out=tp[:], in_=src[:, 128 * k:128 * (k + 1)],
                                identity=ident[:])
            nc.vector.tensor_copy(dst[:, 128 * k:128 * (k + 1)], tp[:])

    v2_ps = pbig.tile([NSEL, H], F32)
    for k in range(4):
        nc.tensor.matmul(v2_ps[:], lhsT=_mm(aT[:, 128 * k:128 * (k + 1)]),
                         rhs=_mm(wvp_sb[:, H * k:H * (k + 1)]),
                         start=(k == 0), stop=False)
    for k in range(4):
        nc.tensor.matmul(v2_ps[:], lhsT=_mm(bT[:, 128 * k:128 * (k + 1)]),
                         rhs=_mm(wtg_sb[:, H * k:H * (k + 1)]),
                         start=False, stop=False)
    nc.tensor.matmul(v2_ps[:], lhsT=_mm(ones1[:]), rhs=_mm(bvt_sb[:]),
                     start=False, stop=True)
    v2 = sb.tile([NSEL, H], F32)
    nc.vector.tensor_add(v2[:], v2_ps[:], pos_sb[:])

    # =============== phase N: cosine sim ===============
    nv = sb.tile([NSEL, 1], F32)
    ja = sb2.tile([NSEL, H], F32)
    nc.vector.tensor_tensor_reduce(out=ja[:], in0=v2[:], in1=v2[:], scale=1.0,
                                   scalar=0.0, op0=OP.mult, op1=OP.add,
                                   accum_out=nv[:])
    nm = sb.tile([NSEL, 1], F32)
    jb = sb2.tile([NSEL, H], F32)
    nc.vector.tensor_tensor_reduce(out=jb[:], in0=v2[:], in1=langB[:], scale=1.0,
                                   scalar=0.0, op0=OP.mult, op1=OP.add,
                                   accum_out=nm[:])
    nl = sb.tile([NSEL, 1], F32)
    jc = sb2.tile([NSEL, H], F32)
    nc.vector.tensor_tensor_reduce(out=jc[:], in0=langB[:], in1=langB[:], scale=1.0,
                                   scalar=0.0, op0=OP.mult, op1=OP.add,
                                   accum_out=nl[:])
    rv = sb.tile([NSEL, 1], F32)
    nc.scalar.activation(rv[:], nv[:], AF.Sqrt)
    rl = sb.tile([NSEL, 1], F32)
    nc.scalar.activation(rl[:], nl[:], AF.Sqrt)
    nc.vector.tensor_scalar_add(rv[:], rv[:], 1e-8)
    nc.vector.tensor_scalar_add(rl[:], rl[:], 1e-8)
    den = sb.tile([NSEL, 1], F32)
    nc.vector.tensor_mul(den[:], rv[:], rl[:])
    rden = sb.tile([NSEL, 1], F32)
    nc.vector.reciprocal(rden[:], den[:])
    simc = sb.tile([NSEL, 1], F32)
    nc.vector.tensor_mul(simc[:], nm[:], rden[:])

    nc.scalar.dma_start(out_sim.rearrange("s j -> (s j) 1"), simc[:])

    # =============== phase F: final box decode ===============
    simdr = dr.tile([NSEL, 1], F32)
    nc.scalar.dma_start(simdr[:], simc[:])
    sim4 = sb.tile([B, SEL], F32)
    nc.scalar.dma_start(sim4[:], simdr[:].rearrange("(s j) one -> s (j one)", s=B))

    m8 = sb.tile([B, 8], F32)
    nc.vector.max(m8[:], sim4[:])
    mk = sb.tile([B, SEL], F32)
    nc.vector.tensor_scalar(mk[:], sim4[:], m8[:, 0:1], None, op0=OP.is_equal)
    selr = sb.tile([B, 1], F32)
    jd = sb2.tile([B, SEL], F32)
    nc.vector.tensor_tensor_reduce(out=jd[:], in0=mk[:], in1=gidxf[:], scale=1.0,
                                   scalar=0.0, op0=OP.mult, op1=OP.add,
                                   accum_out=selr[:])
    offs3 = sb.tile([B, 1], I32)
    nc.vector.tensor_copy(offs3[:], selr[:])

    bsel4 = sb.tile([B, A * CH], F32)
    nc.gpsimd.indirect_dma_start(
        out=bsel4[:], out_offset=None, in_=boxes_r,
        in_offset=bass.IndirectOffsetOnAxis(ap=offs3[:, :1], axis=0))

    bv = bsel4[:].rearrange("p (a c) -> p a c", a=A)
    val = sb.tile([B, A * 5], F32)
    vv = val[:].rearrange("p (a c) -> p a c", a=A)
    w2 = sb.tile([B, A], F32)
    nc.vector.tensor_scalar(w2[:], bv[:, :, 2], 0.5, None, op0=OP.mult)
    h2 = sb.tile([B, A], F32)
    nc.vector.tensor_scalar(h2[:], bv[:, :, 3], 0.5, None, op0=OP.mult)
    nc.vector.tensor_sub(vv[:, :, 0], bv[:, :, 0], w2[:])
    nc.vector.tensor_sub(vv[:, :, 1], bv[:, :, 1], h2[:])
    nc.vector.tensor_add(vv[:, :, 2], vv[:, :, 0], bv[:, :, 2])
    nc.vector.tensor_add(vv[:, :, 3], vv[:, :, 1], bv[:, :, 3])
    nc.vector.tensor_copy(vv[:, :, 4], bv[:, :, 4])

    mo = sb.tile([B, 1], F32)
    nc.vector.tensor_reduce(mo[:], bv[:, :, 4], axis=mybir.AxisListType.X,
                            op=OP.max)
    mk3 = sb.tile([B, A], F32)
    nc.vector.tensor_scalar(mk3[:], bv[:, :, 4], mo[:, 0:1], None,
                            op0=OP.is_equal)

    pm = sb.tile([B, 5 * A], F32)
    nc.vector.tensor_tensor(
        pm[:].rearrange("p (c a) -> p c a", a=A),
        val[:].rearrange("p (a c) -> p c a", a=A),
        mk3[:, None, :].to_broadcast([B, 5, A]),
        op=OP.mult)
    bp = sb.tile([B, 5], F32)
    nc.vector.tensor_reduce(bp[:], pm[:].rearrange("p (c a) -> p c a", a=A),
                            axis=mybir.AxisListType.X, op=OP.add)
    nc.scalar.dma_start(io["out_box"], bp[:])

    for p in reversed(ctx_pools):
        p.__exit__(None, None, None)


def build_program(loop_k=None):
    """Build + compile the per-core program. loop_k wraps the body in a
    dynamic For_i loop (used only for timing measurements)."""
    nc = bacc.Bacc("TRN2", target_bir_lowering=False, debug=False)
    with tile.TileContext(nc) as tc:
        if loop_k is None:
            emit(tc)
        else:
            with tc.For_i(0, loop_k, 1):
                emit(tc)
    nc.compile()
    return nc


# ---------------- host side ----------------

def _shard_inputs(inputs):
    """Full inputs -> list of 8 per-core input dicts (layout prep only)."""
    boxes = np.ascontiguousarray(inputs["boxes"], dtype=np.float32)
    x_feat = np.ascontiguousarray(inputs["x_feat"], dtype=np.float32)
    tag_emb = np.asarray(inputs["tag_emb"], dtype=np.float32)
    pos_emb = np.asarray(inputs["pos_emb"], dtype=np.float32)
    lang = np.asarray(inputs["lang"], dtype=np.float32)

    obj_full = np.ascontiguousarray(boxes[:, :, :, 4].transpose(0, 2, 1))  # [BS, A, GRID]
    xf_full = np.ascontiguousarray(
        x_feat.reshape(BS, V, GRID).transpose(0, 2, 1))  # [BS, GRID, V]

    shared = {
        "W_vs": np.ascontiguousarray(inputs["W_vs"], np.float32),
        "W_ts": np.ascontiguousarray(inputs["W_ts"], np.float32),
        "W_vp": np.ascontiguousarray(inputs["W_vs_pos"], np.float32),
        "W_tg": np.ascontiguousarray(inputs["W_tag"], np.float32),
        "W_soft": np.ascontiguousarray(inputs["W_soft"], np.float32),
        "b_vs": np.ascontiguousarray(inputs["b_vs"], np.float32),
        "b_ts": np.ascontiguousarray(inputs["b_ts"], np.float32),
        "b_vp": np.ascontiguousarray(inputs["b_vs_pos"], np.float32),
        "b_tg": np.ascontiguousarray(inputs["b_tag"], np.float32),
        "b_soft": np.ascontiguousarray(inputs["b_soft"], np.float32),
        "bsel64": (np.arange(P64)[None, :] // C16 ==
                   np.arange(B)[:, None]).astype(np.float32),
        "bsel128": (np.arange(NSEL)[None, :] // SEL ==
                    np.arange(B)[:, None]).astype(np.float32),
    }
    in_maps = []
    for c in range(N_CORES):
        s = slice(c * B, (c + 1) * B)
        m = dict(shared)
        m["obj_t"] = obj_full[s]
        m["boxes_r"] = boxes[s].reshape(FLATG, A * CH)
        m["xf_t"] = xf_full[s].reshape(FLATG, V)
        m["tag_r"] = tag_emb[s].reshape(NSEL, H)
        m["pos_r"] = pos_emb[s].reshape(NSEL, H)
        m["langT"] = np.ascontiguousarray(lang[s].T)
        in_maps.append(m)
    return in_maps


_CACHED_NC = None


def kernel(**inputs):
    global _CACHED_NC
    from concourse.bass_utils import run_bass_kernel_spmd
    if _CACHED_NC is None:
        _CACHED_NC = build_program()
    in_maps = _shard_inputs(inputs)
    res = run_bass_kernel_spmd(_CACHED_NC, in_maps, list(range(N_CORES)))
    box = np.concatenate([res.results[c]["out_box"] for c in range(N_CORES)], 0)
    sim = np.concatenate([res.results[c]["out_sim"] for c in range(N_CORES)], 0)
    return box.reshape(BS, 1, 5), sim


# revision 13
# speedup vs baseline: 1.2255x; 1.2255x over previous
"""Trainium2 Bass kernel for nn_Net_5128190952056 (topk_masking).

Data-parallel over batch: 32 samples -> 8 NeuronCores x 4 samples.
Per-core pipeline (all compute on device):
  score = sum_a boxes[:, :, a, 4]  ->  exact top-32 grid selection (DVE
  max8/match_replace hierarchy + max_index, jax.top_k-compatible set)  ->
  sorted-index extraction  ->  indirect-DMA gathers of selected features /
  boxes  ->  gated multimodal fusion (PE matmuls)  ->  cosine-sim head  ->
  per-sample argmax box decode.
"""
import os
import sys
sys.path.insert(0, "/opt/trn_rl_repo")

import numpy as np

import concourse.bass as bass
import concourse.tile as tile
from concourse import bacc, mybir
from concourse.masks import make_identity

F32 = mybir.dt.float32
I32 = mybir.dt.int32
U32 = mybir.dt.uint32
AF = mybir.ActivationFunctionType
OP = mybir.AluOpType

N_CORES = 8
BS = 32
B = BS // N_CORES   # per-core batch = 4
GRID = 2704
A = 3
CH = 85
V = 1024
H = 512
SEL = 32
NSEL = B * SEL      # 128
C16 = 16            # score chunks per sample
F169 = GRID // C16  # 169
P64 = B * C16       # 64 partitions for score layout
FLATG = B * GRID    # 10816
KEYC = 32768.0      # key offset for index extraction

MM_DT = mybir.dt.float32
STOP_PHASE = int(os.environ.get("STOP_PHASE", "99"))
ACT_RED = int(os.environ.get("ACT_RED", "0"))  # reductions on ACT engine


def _mm(ap):
    return ap.bitcast(MM_DT) if MM_DT != F32 else ap


def emit(tc):
    nc = tc.nc

    def dram(name, shape):
        return nc.dram_tensor(name, shape, F32, kind="ExternalInput").ap()

    obj_t = dram("obj_t", [P64, A * F169])
    boxes_r = dram("boxes_r", [FLATG, A * CH])
    xf_t = dram("xf_t", [FLATG, V])
    tag_r = dram("tag_r", [NSEL, H])
    pos_r = dram("pos_r", [NSEL, H])
    langT = dram("langT", [H, B])
    W_vs = dram("W_vs", [V, H])
    W_ts = dram("W_ts", [H, H])
    W_vp = dram("W_vp", [H, H])
    W_tg = dram("W_tg", [H, H])
    W_soft = dram("W_soft", [H, 2])
    b_vs = dram("b_vs", [H])
    b_ts = dram("b_ts", [H])
    b_vp = dram("b_vp", [H])
    b_tg = dram("b_tg", [H])
    b_soft = dram("b_soft", [2])
    bsel128 = dram("bsel128", [B, NSEL])
    out_sim = nc.dram_tensor("out_sim", [B, SEL], F32, kind="ExternalOutput").ap()
    out_box = nc.dram_tensor("out_box", [B, 5], F32, kind="ExternalOutput").ap()
    return _emit_body(tc, locals())


def _emit_body(tc, io):
    nc = tc.nc
    ctx_pools = []

    sb = tc.alloc_tile_pool(name="sb", bufs=1)
    sb2 = tc.alloc_tile_pool(name="sb2", bufs=2)
    pbig = tc.alloc_tile_pool(name="pbig", bufs=1, space="PSUM")
    ptp = tc.alloc_tile_pool(name="ptp", bufs=2, space="PSUM")
    psm = tc.alloc_tile_pool(name="psm", bufs=2, space="PSUM")
    ctx_pools += [sb, sb2, pbig, ptp, psm]

    obj_t, boxes_r, xf_t, tag_r, pos_r, langT = (
        io["obj_t"], io["boxes_r"], io["xf_t"], io["tag_r"], io["pos_r"], io["langT"])
    W_vs, W_ts, W_vp, W_tg, W_soft = (
        io["W_vs"], io["W_ts"], io["W_vp"], io["W_tg"], io["W_soft"])
    b_vs, b_ts, b_vp, b_tg, b_soft = (
        io["b_vs"], io["b_ts"], io["b_vp"], io["b_tg"], io["b_soft"])
    bsel128, out_sim, out_box = (io["bsel128"], io["out_sim"], io["out_box"])

    def stop_here(dump_ap=None):
        if dump_ap is not None:
            nc.scalar.dma_start(out_sim, dump_ap)
        zb = sb.tile([B, 5], F32)
        nc.vector.memset(zb[:], 0.0)
        nc.scalar.dma_start(out_box, zb[:])
        for p in reversed(ctx_pools):
            p.release()

    # =============== phase S: score + exact top-32 selection ===============
    obj_sb = sb.tile([P64, A * F169], F32)
    nc.sync.dma_start(obj_sb[:], obj_t)

    s0 = sb.tile([P64, F169], F32)
    nc.vector.tensor_add(s0[:], obj_sb[:, 0:F169], obj_sb[:, F169:2 * F169])
    nc.vector.tensor_add(s0[:], s0[:], obj_sb[:, 2 * F169:3 * F169])

    # per-partition top-32 candidates: values + local indices
    vt = sb.tile([P64, F169], F32)
    cand = sb.tile([P64, SEL], F32)
    cidx = sb.tile([P64, SEL], U32)
    for r in range(4):
        src = s0 if r == 0 else vt
        nc.vector.max(cand[:, 8 * r:8 * r + 8], src[:])
        nc.vector.max_index(cidx[:, 8 * r:8 * r + 8], cand[:, 8 * r:8 * r + 8],
                            src[:])
        nc.vector.match_replace(vt[:], cand[:, 8 * r:8 * r + 8], src[:], -1.0)

    # candidate flat row index = local_f + 169 * partition
    poffi = sb.tile([P64, 1], I32)
    nc.gpsimd.iota(poffi[:], pattern=[[1, 1]], base=0, channel_multiplier=F169)
    poff = sb.tile([P64, 1], F32)
    nc.vector.tensor_copy(poff[:], poffi[:])
    cidx_f = sb.tile([P64, SEL], F32)
    nc.vector.tensor_copy(cidx_f[:], cidx[:])
    gidxc = sb.tile([P64, SEL], F32)
    nc.vector.tensor_scalar(gidxc[:], cidx_f[:], poff[:, 0:1], None, op0=OP.add)

    # merge to per-sample [4, 512] (direct SBUF->SBUF partition restructure)
    vm = sb.tile([B, C16 * SEL], F32)
    nc.scalar.dma_start(vm[:], cand[:])
    cm = sb.tile([B, C16 * SEL], F32)
    nc.scalar.dma_start(cm[:], gidxc[:])

    gtop = sb.tile([B, SEL], F32)
    vmx = sb.tile([B, C16 * SEL], F32)
    for r in range(4):
        src = vm if r == 0 else vmx
        nc.vector.max(gtop[:, 8 * r:8 * r + 8], src[:])
        nc.vector.match_replace(vmx[:], gtop[:, 8 * r:8 * r + 8], src[:], -1.0)

    # selected = candidates >= 32nd-largest value; extract ascending flat idx
    mask512 = sb.tile([B, C16 * SEL], F32)
    nc.vector.tensor_scalar(mask512[:], vm[:], gtop[:, SEL - 1:SEL], None,
                            op0=OP.is_ge)
    keyt = sb.tile([B, C16 * SEL], F32)
    nc.vector.tensor_scalar(keyt[:], cm[:], KEYC, -1.0,
                            op0=OP.subtract, op1=OP.mult)
    key512 = sb.tile([B, C16 * SEL], F32)
    nc.vector.tensor_tensor(key512[:], keyt[:], mask512[:], op=OP.mult)
    gk = sb.tile([B, SEL], F32)
    for r in range(4):
        nc.vector.max(gk[:, 8 * r:8 * r + 8], key512[:])
        nc.vector.match_replace(key512[:], gk[:, 8 * r:8 * r + 8], key512[:], 0.0)

    gidxf = sb.tile([B, SEL], F32)
    nc.vector.tensor_scalar(gidxf[:], gk[:], KEYC, -1.0,
                            op0=OP.subtract, op1=OP.mult)
    gidxi = sb.tile([B, SEL], I32)
    nc.vector.tensor_copy(gidxi[:], gidxf[:])
    offs = sb.tile([NSEL, 1], I32)
    nc.scalar.dma_start(offs[:], gidxi[:])

    if STOP_PHASE <= 1:
        return stop_here(gidxf[:])

    # =============== phase G: gather selected visual features ===============
    isel = sb.tile([NSEL, V], F32)
    nc.gpsimd.indirect_dma_start(
        out=isel[:], out_offset=None, in_=xf_t,
        in_offset=bass.IndirectOffsetOnAxis(ap=offs[:, :1], axis=0))

    # =============== weights / constants ===============
    wvs_sb = sb.tile([128, 8 * H], F32)
    nc.sync.dma_start(wvs_sb[:].rearrange("p (k h) -> p k h", k=8),
                      W_vs.rearrange("(k p) h -> p k h", p=128))
    wts_sb = sb.tile([128, 4 * H], F32)
    nc.sync.dma_start(wts_sb[:].rearrange("p (k h) -> p k h", k=4),
                      W_ts.rearrange("(k p) h -> p k h", p=128))
    wvp_sb = sb.tile([128, 4 * H], F32)
    nc.sync.dma_start(wvp_sb[:].rearrange("p (k h) -> p k h", k=4),
                      W_vp.rearrange("(k p) h -> p k h", p=128))
    wtg_sb = sb.tile([128, 4 * H], F32)
    nc.sync.dma_start(wtg_sb[:].rearrange("p (k h) -> p k h", k=4),
                      W_tg.rearrange("(k p) h -> p k h", p=128))
    wsoft_sb = sb.tile([1, H * 2], F32)
    nc.sync.dma_start(wsoft_sb[:], W_soft.rearrange("h c -> (h c)")[None, :])
    tag_sb = sb.tile([NSEL, H], F32)
    nc.sync.dma_start(tag_sb[:], tag_r)
    pos_sb = sb.tile([NSEL, H], F32)
    nc.sync.dma_start(pos_sb[:], pos_r)
    langT_sb = sb.tile([128, 4 * B], F32)
    nc.sync.dma_start(langT_sb[:].rearrange("p (k b) -> p k b", k=4),
                      langT.rearrange("(k p) b -> p k b", p=128))
    bsel128_sb = sb.tile([B, NSEL], F32)
    nc.sync.dma_start(bsel128_sb[:], bsel128)

    bvs_sb = sb.tile([1, H], F32)
    nc.sync.dma_start(bvs_sb[:], b_vs[None, :])
    bts_sb = sb.tile([1, H], F32)
    nc.sync.dma_start(bts_sb[:], b_ts[None, :])
    bvp_sb = sb.tile([1, H], F32)
    nc.sync.dma_start(bvp_sb[:], b_vp[None, :])
    btg_sb = sb.tile([1, H], F32)
    nc.sync.dma_start(btg_sb[:], b_tg[None, :])
    bvt_sb = sb.tile([1, H], F32)
    nc.vector.tensor_add(bvt_sb[:], bvp_sb[:], btg_sb[:])
    bsoft_sb = sb.tile([1, 2], F32)
    nc.sync.dma_start(bsoft_sb[:], b_soft[None, :])

    ones1 = sb.tile([1, 128], F32)
    nc.vector.memset(ones1[:], 1.0)
    ident = sb.tile([128, 128], F32)
    make_identity(nc, ident[:])

    # =============== phase T: transpose gathered features ===============
    itk = sb.tile([128, 8 * 128], F32)
    for k in range(8):
        tp = ptp.tile([128, 128], F32)
        nc.tensor.transpose(out=tp[:], in_=isel[:, 128 * k:128 * (k + 1)],
                            identity=ident[:])
        nc.vector.tensor_copy(itk[:, 128 * k:128 * (k + 1)], tp[:])

    # =============== phase M1: vis = isel @ W_vs + b_vs ===============
    vis_ps = pbig.tile([NSEL, H], F32, tag="mm_a")
    for k in range(8):
        nc.tensor.matmul(vis_ps[:], lhsT=_mm(itk[:, 128 * k:128 * (k + 1)]),
                         rhs=_mm(wvs_sb[:, H * k:H * (k + 1)]),
                         start=(k == 0), stop=False)
    nc.tensor.matmul(vis_ps[:], lhsT=_mm(ones1[:]), rhs=_mm(bvs_sb[:]),
                     start=False, stop=True)
    vis = sb.tile([NSEL, H], F32)
    nc.vector.tensor_copy(vis[:], vis_ps[:])

    if STOP_PHASE <= 2:
        return stop_here(vis[0:B, 0:SEL])

    # =============== phase M2: lang_e, broadcast per sample ===============
    le_ps = psm.tile([B, H], F32, tag="psmall")
    for k in range(4):
        nc.tensor.matmul(le_ps[:], lhsT=_mm(langT_sb[:, B * k:B * (k + 1)]),
                         rhs=_mm(wts_sb[:, H * k:H * (k + 1)]),
                         start=(k == 0), stop=False)
    nc.tensor.matmul(le_ps[:], lhsT=_mm(ones1[:, 0:B]), rhs=_mm(bts_sb[:]),
                     start=False, stop=True)
    le = sb.tile([B, H], F32)
    nc.vector.tensor_copy(le[:], le_ps[:])
    langB_ps = pbig.tile([NSEL, H], F32, tag="mm_b")
    nc.tensor.matmul(langB_ps[:], lhsT=_mm(bsel128_sb[:]), rhs=_mm(le[:]),
                     start=True, stop=True)
    langB = sb.tile([NSEL, H], F32)
    nc.vector.tensor_copy(langB[:], langB_ps[:])

    # =============== phase M3: logits + log-softmax weights ===============
    xs = sb.tile([NSEL, H], F32)
    nc.vector.tensor_add(xs[:], vis[:], tag_sb[:])
    nc.vector.tensor_add(xs[:], xs[:], pos_sb[:])

    bs_ps = psm.tile([NSEL, 2], F32, tag="psmall")
    nc.tensor.matmul(bs_ps[:], lhsT=_mm(ones1[:]), rhs=_mm(bsoft_sb[:]),
                     start=True, stop=True)
    bs128 = sb.tile([NSEL, 2], F32)
    nc.vector.tensor_copy(bs128[:], bs_ps[:])

    wsb = []
    for c in range(2):
        w_ps = pbig.tile([NSEL, H], F32, tag="mm_c")
        nc.tensor.matmul(w_ps[:], lhsT=_mm(ones1[:]),
                         rhs=_mm(wsoft_sb[:].rearrange("one (h c) -> one c h",
                                                       c=2)[:, c, :]),
                         start=True, stop=True)
        w_sb = sb.tile([NSEL, H], F32, tag=f"wsb{c}")
        nc.vector.tensor_copy(w_sb[:], w_ps[:])
        wsb.append(w_sb)

    def dotred(prod_in0, prod_in1, accum):
        pj = sb2.tile([NSEL, H], F32, tag="prod")
        nc.vector.tensor_mul(pj[:], prod_in0, prod_in1)
        if ACT_RED:
            po = sb2.tile([NSEL, H], F32, tag="acc_o")
            nc.scalar.activation(po[:], pj[:], AF.Identity, accum_out=accum)
        else:
            nc.vector.tensor_reduce(accum, pj[:], axis=mybir.AxisListType.X,
                                    op=OP.add)

    lg0 = sb.tile([NSEL, 1], F32)
    dotred(xs[:], wsb[0][:], lg0[:])
    nc.vector.tensor_add(lg0[:], lg0[:], bs128[:, 0:1])
    lg1 = sb.tile([NSEL, 1], F32)
    dotred(xs[:], wsb[1][:], lg1[:])
    nc.vector.tensor_add(lg1[:], lg1[:], bs128[:, 1:2])

    d = sb.tile([NSEL, 1], F32)
    nc.vector.tensor_sub(d[:], lg0[:], lg1[:])
    # w0 = -softplus(-ds), w1 = -softplus(ds), ds = d/0.03; stable form
    # softplus(x) = relu(x) + log1p(exp(-|x|)) via the exp/ln ACT table.
    TS = 1.0 / 0.03
    ad = sb.tile([NSEL, 1], F32)
    nc.scalar.activation(ad[:], d[:], AF.Abs)
    e = sb.tile([NSEL, 1], F32)
    nc.scalar.activation(e[:], ad[:], AF.Exp, scale=-TS)
    l1p = sb.tile([NSEL, 1], F32)
    nc.scalar.activation(l1p[:], e[:], AF.Ln, bias=1.0)
    rpos = sb.tile([NSEL, 1], F32)
    nc.scalar.activation(rpos[:], d[:], AF.Relu, scale=TS)
    rneg = sb.tile([NSEL, 1], F32)
    nc.scalar.activation(rneg[:], d[:], AF.Relu, scale=-TS)
    w0 = sb.tile([NSEL, 1], F32)
    nc.vector.scalar_tensor_tensor(w0[:], in0=rneg[:], scalar=-1.0, in1=l1p[:],
                                   op0=OP.mult, op1=OP.subtract)
    w1 = sb.tile([NSEL, 1], F32)
    nc.vector.scalar_tensor_tensor(w1[:], in0=rpos[:], scalar=-1.0, in1=l1p[:],
                                   op0=OP.mult, op1=OP.subtract)

    aS = sb.tile([NSEL, H], F32)
    nc.vector.tensor_scalar(aS[:], vis[:], w0[:, 0:1], None, op0=OP.mult)
    bS = sb.tile([NSEL, H], F32)
    nc.vector.tensor_scalar(bS[:], tag_sb[:], w1[:, 0:1], None, op0=OP.mult)

    # =============== phase T2 + M4: vis2 ===============
    aT = sb.tile([128, 4 * 128], F32)
    bT = sb.tile([128, 4 * 128], F32)
    for src, dst in ((aS, aT), (bS, bT)):
        for k in range(4):
            tp = ptp.tile([128, 128], F32)
            nc.tensor.transpose(out=tp[:], in_=src[:, 128 * k:128 * (k + 1)],
                                identity=ident[:])
            nc.vector.tensor_copy(dst[:, 128 * k:128 * (k + 1)], tp[:])

    v2_ps = pbig.tile([NSEL, H], F32, tag="mm_a")
    for k in range(4):
        nc.tensor.matmul(v2_ps[:], lhsT=_mm(aT[:, 128 * k:128 * (k + 1)]),
                         rhs=_mm(wvp_sb[:, H * k:H * (k + 1)]),
                         start=(k == 0), stop=False)
    for k in range(4):
        nc.tensor.matmul(v2_ps[:], lhsT=_mm(bT[:, 128 * k:128 * (k + 1)]),
                         rhs=_mm(wtg_sb[:, H * k:H * (k + 1)]),
                         start=False, stop=False)
    nc.tensor.matmul(v2_ps[:], lhsT=_mm(ones1[:]), rhs=_mm(bvt_sb[:]),
                     start=False, stop=True)
    v2 = sb.tile([NSEL, H], F32)
    nc.vector.tensor_add(v2[:], v2_ps[:], pos_sb[:])

    # =============== phase N: cosine sim (reductions on ACT) ===============
    nv = sb.tile([NSEL, 1], F32)
    nl = sb.tile([NSEL, 1], F32)
    if ACT_RED:
        ja = sb2.tile([NSEL, H], F32, tag="acc_o")
        nc.scalar.activation(ja[:], v2[:], AF.Square, accum_out=nv[:])
        jc = sb2.tile([NSEL, H], F32, tag="acc_o")
        nc.scalar.activation(jc[:], langB[:], AF.Square, accum_out=nl[:])
    else:
        dotred(v2[:], v2[:], nv[:])
        dotred(langB[:], langB[:], nl[:])
    nm = sb.tile([NSEL, 1], F32)
    dotred(v2[:], langB[:], nm[:])

    # sim = nm / (sqrt(nv)*sqrt(nl)) = nm * exp(-0.5*(ln(nv)+ln(nl)))
    lnv = sb.tile([NSEL, 1], F32)
    nc.scalar.activation(lnv[:], nv[:], AF.Ln)
    lnl = sb.tile([NSEL, 1], F32)
    nc.scalar.activation(lnl[:], nl[:], AF.Ln)
    lsum = sb.tile([NSEL, 1], F32)
    nc.vector.tensor_add(lsum[:], lnv[:], lnl[:])
    rinv = sb.tile([NSEL, 1], F32)
    nc.scalar.activation(rinv[:], lsum[:], AF.Exp, scale=-0.5)
    simc = sb.tile([NSEL, 1], F32)
    nc.vector.tensor_mul(simc[:], nm[:], rinv[:])

    nc.scalar.dma_start(out_sim.rearrange("s j -> (s j)")[:, None], simc[:])

    # =============== phase F: final box decode ===============
    sim4 = sb.tile([B, SEL], F32)
    nc.scalar.dma_start(sim4[:], simc[:])

    m8 = sb.tile([B, 8], F32)
    nc.vector.max(m8[:], sim4[:])
    mk = sb.tile([B, SEL], F32)
    nc.vector.tensor_scalar(mk[:], sim4[:], m8[:, 0:1], None, op0=OP.is_equal)
    selr = sb.tile([B, 1], F32)
    jd = sb2.tile([B, SEL], F32)
    nc.vector.tensor_mul(jd[:], mk[:], gidxf[:])
    nc.vector.tensor_reduce(selr[:], jd[:], axis=mybir.AxisListType.X, op=OP.add)
    offs3 = sb.tile([B, 1], I32)
    nc.vector.tensor_copy(offs3[:], selr[:])

    bsel4 = sb.tile([B, A * CH], F32)
    nc.gpsimd.indirect_dma_start(
        out=bsel4[:], out_offset=None, in_=boxes_r,
        in_offset=bass.IndirectOffsetOnAxis(ap=offs3[:, :1], axis=0))

    bv = bsel4[:].rearrange("p (a c) -> p a c", a=A)
    val = sb.tile([B, A * 5], F32)
    vv = val[:].rearrange("p (a c) -> p a c", a=A)
    w2 = sb.tile([B, A], F32)
    nc.vector.tensor_scalar(w2[:], bv[:, :, 2], 0.5, None, op0=OP.mult)
    h2 = sb.tile([B, A], F32)
    nc.vector.tensor_scalar(h2[:], bv[:, :, 3], 0.5, None, op0=OP.mult)
    nc.vector.tensor_sub(vv[:, :, 0], bv[:, :, 0], w2[:])
    nc.vector.tensor_sub(vv[:, :, 1], bv[:, :, 1], h2[:])
    nc.vector.tensor_add(vv[:, :, 2], vv[:, :, 0], bv[:, :, 2])
    nc.vector.tensor_add(vv[:, :, 3], vv[:, :, 1], bv[:, :, 3])
    nc.vector.tensor_copy(vv[:, :, 4], bv[:, :, 4])

    mo = sb.tile([B, 1], F32)
    nc.vector.tensor_reduce(mo[:], bv[:, :, 4], axis=mybir.AxisListType.X,
                            op=OP.max)
    mk3 = sb.tile([B, A], F32)
    nc.vector.tensor_scalar(mk3[:], bv[:, :, 4], mo[:, 0:1], None,
                            op0=OP.is_equal)

    pm = sb.tile([B, 5 * A], F32)
    nc.vector.tensor_tensor(
        pm[:].rearrange("p (c a) -> p c a", a=A),
        val[:].rearrange("p (a c) -> p c a", a=A),
        mk3[:, None, :].to_broadcast([B, 5, A]),
        op=OP.mult)
    bp = sb.tile([B, 5], F32)
    nc.vector.tensor_reduce(bp[:], pm[:].rearrange("p (c a) -> p c a", a=A),
                            axis=mybir.AxisListType.X, op=OP.add)
    nc.scalar.dma_start(out_box, bp[:])

    for p in reversed(ctx_pools):
        p.release()


def build_program(loop_k=None):
    nc = bacc.Bacc("TRN2", target_bir_lowering=False, debug=False)
    with tile.TileContext(nc) as tc:
        if loop_k is None:
            emit(tc)
        else:
            with tc.For_i(0, loop_k, 1):
                emit(tc)
    nc.compile()
    return nc


# ---------------- host side ----------------

def _shard_inputs(inputs):
    """Full inputs -> list of 8 per-core input dicts (layout prep only)."""
    boxes = np.ascontiguousarray(inputs["boxes"], dtype=np.float32)
    x_feat = np.ascontiguousarray(inputs["x_feat"], dtype=np.float32)
    tag_emb = np.asarray(inputs["tag_emb"], dtype=np.float32)
    pos_emb = np.asarray(inputs["pos_emb"], dtype=np.float32)
    lang = np.asarray(inputs["lang"], dtype=np.float32)

    obj_full = np.ascontiguousarray(boxes[:, :, :, 4].transpose(0, 2, 1))
    xf_full = np.ascontiguousarray(
        x_feat.reshape(BS, V, GRID).transpose(0, 2, 1))

    shared = {
        "W_vs": np.ascontiguousarray(inputs["W_vs"], np.float32),
        "W_ts": np.ascontiguousarray(inputs["W_ts"], np.float32),
        "W_vp": np.ascontiguousarray(inputs["W_vs_pos"], np.float32),
        "W_tg": np.ascontiguousarray(inputs["W_tag"], np.float32),
        "W_soft": np.ascontiguousarray(inputs["W_soft"], np.float32),
        "b_vs": np.ascontiguousarray(inputs["b_vs"], np.float32),
        "b_ts": np.ascontiguousarray(inputs["b_ts"], np.float32),
        "b_vp": np.ascontiguousarray(inputs["b_vs_pos"], np.float32),
        "b_tg": np.ascontiguousarray(inputs["b_tag"], np.float32),
        "b_soft": np.ascontiguousarray(inputs["b_soft"], np.float32),
        "bsel128": (np.arange(NSEL)[None, :] // SEL ==
                    np.arange(B)[:, None]).astype(np.float32),
    }
    in_maps = []
    for c in range(N_CORES):
        s = slice(c * B, (c + 1) * B)
        m = dict(shared)
        m["obj_t"] = np.ascontiguousarray(
            obj_full[s].reshape(B, A, C16, F169).transpose(0, 2, 1, 3)
        ).reshape(P64, A * F169)
        m["boxes_r"] = boxes[s].reshape(FLATG, A * CH)
        m["xf_t"] = xf_full[s].reshape(FLATG, V)
        m["tag_r"] = tag_emb[s].reshape(NSEL, H)
        m["pos_r"] = pos_emb[s].reshape(NSEL, H)
        m["langT"] = np.ascontiguousarray(lang[s].T)
        in_maps.append(m)
    return in_maps


_CACHED_NC = None


def kernel(**inputs):
    global _CACHED_NC
    from concourse.bass_utils import run_bass_kernel_spmd
    if _CACHED_NC is None:
        _CACHED_NC = build_program()
    in_maps = _shard_inputs(inputs)
    res = run_bass_kernel_spmd(_CACHED_NC, in_maps, list(range(N_CORES)))
    box = np.concatenate([res.results[c]["out_box"] for c in range(N_CORES)], 0)
    sim = np.concatenate([res.results[c]["out_sim"] for c in range(N_CORES)], 0)
    return box.reshape(BS, 1, 5), sim
